# revision 1
# baseline (speedup 1.0000x reference)
"""Multi-head attention (B=2, S=2048, D=2048, H=16) on 8 Trainium2 cores.

Sharding: core = batch (2) x head-group (4 heads each). Tensor-parallel on
wq/wk/wv rows + wo columns; per-core partial outputs summed on host.

Device kernel (per core, all matmuls in float32r):
  phase 1: qT/kT (head_dim, seq) + v (seq, head_dim) projections, RoPE on q/k
  phase 2: scores^T -> exp -> denominator (ones-matmul) + attn@V, normalize
  phase 3: output projection partial (seq, dim)
"""

import sys

for _p in ("/opt/trn_rl_repo",):
    if _p not in sys.path:
        sys.path.insert(0, _p)

import numpy as np

import concourse.bass as bass
import concourse.tile as tile
from concourse import bacc, mybir
from concourse.bass_utils import run_bass_kernel_spmd

F32 = mybir.dt.float32
F32R = mybir.dt.float32r

DIM = 2048
N_HEADS = 16
HEAD_DIM = 128
BATCH = 2
SEQ = 2048
G_HEADS = 4          # heads per core
GM = G_HEADS * HEAD_DIM  # 512 output cols per core
DC = DIM // 128      # 16 contraction chunks
SC512 = SEQ // 512   # 4
SC128 = SEQ // 128   # 16
INV_SQRT_HD = float(1.0 / np.sqrt(HEAD_DIM))

# even<->odd partition swap within each 32-partition group
_SWAP_MASK = [i ^ 1 for i in range(32)]


def build(with_mask: bool):
    nc = bacc.Bacc("TRN2", target_bir_lowering=False, debug=False)

    xt_d = nc.dram_tensor("xt", [DC, 128, SEQ], F32R, kind="ExternalInput").ap()
    wq_d = nc.dram_tensor("wq", [DC, G_HEADS, 128, 128], F32R, kind="ExternalInput").ap()
    wk_d = nc.dram_tensor("wk", [DC, G_HEADS, 128, 128], F32R, kind="ExternalInput").ap()
    wv_d = nc.dram_tensor("wv", [DC, 128, GM], F32R, kind="ExternalInput").ap()
    wo_d = nc.dram_tensor("wo", [G_HEADS, 128, SEQ], F32R, kind="ExternalInput").ap()
    ce_d = nc.dram_tensor("ce", [128, SEQ], F32R, kind="ExternalInput").ap()
    s2_d = nc.dram_tensor("s2", [128, SEQ], F32R, kind="ExternalInput").ap()
    ones_d = nc.dram_tensor("ones", [128, 1], F32R, kind="ExternalInput").ap()
    mt_d = None
    if with_mask:
        mt_d = nc.dram_tensor("mt", [SC128, 128, SEQ], F32, kind="ExternalInput").ap()
    out_d = nc.dram_tensor("out", [SC128, 128, SEQ], F32, kind="ExternalOutput").ap()

    with tile.TileContext(nc) as tc:
        with (
            tc.tile_pool(name="persist", bufs=1) as persist,
            tc.tile_pool(name="consts", bufs=1) as consts,
        ):
            ones_t = consts.tile([128, 1], F32R, tag="ones")
            nc.sync.dma_start(ones_t[:], ones_d)
            # warm the ACT exp LUT early so phase 2 doesn't stall on it
            warm_t = consts.tile([128, 1], F32, tag="warm")
            nc.scalar.activation(
                out=warm_t[:], in_=ones_t[:],
                func=mybir.ActivationFunctionType.Exp,
            )

            q_t = [persist.tile([128, SEQ], F32R, tag=f"q{h}", name=f"q{h}") for h in range(G_HEADS)]
            k_t = [persist.tile([128, SEQ], F32R, tag=f"k{h}", name=f"k{h}") for h in range(G_HEADS)]
            v_t = [persist.tile([128, GM], F32R, tag=f"v{s}", name=f"v{s}") for s in range(SC128)]

            # ---------------- phase 1: projections + fused rope ----------------
            with (
                tc.tile_pool(name="rope_c", bufs=1) as rope_c,
                tc.tile_pool(name="xt", bufs=8) as xt_pool,
                tc.tile_pool(name="wqk", bufs=2) as wqk_pool,
                tc.tile_pool(name="wv", bufs=1) as wv_pool,
                tc.tile_pool(name="ps1", bufs=4, space="PSUM") as ps1,
                tc.tile_pool(name="rope_t", bufs=1) as rope_t,
            ):
                ce_t = rope_c.tile([128, SEQ], F32R, tag="ce")
                s2_t = rope_c.tile([128, SEQ], F32R, tag="s2")

                def rope(t, sl):
                    # sin-product on the otherwise-idle GpSimd engine
                    t1 = rope_t.tile([128, 512], F32, tag="t1", name="t1")
                    nc.gpsimd.tensor_mul(out=t1[:], in0=t[:, sl], in1=s2_t[:, sl])
                    t2 = rope_t.tile([128, 512], F32, tag="t2", name="t2")
                    nc.vector.stream_shuffle(t2[:], t1[:], _SWAP_MASK)
                    t3 = rope_t.tile([128, 512], F32, tag="t3", name="t3")
                    nc.vector.tensor_mul(out=t3[:], in0=t[:, sl], in1=ce_t[:, sl])
                    nc.vector.tensor_add(out=t[:, sl], in0=t3[:], in1=t2[:])

                for half in range(2):
                    dcs = list(range(half * 8, half * 8 + 8))
                    # first head's weights + first-half x tiles are what the
                    # very first matmul chain needs: pin them to the front of
                    # the scheduler's priority heap so no other dep-free DMA
                    # (wvt/ce/s2/...) gets hoisted ahead of them
                    prio = tc.high_priority() if half == 0 else None
                    if prio is not None:
                        prio.__enter__()
                    wt_first = wqk_pool.tile([128, 8, 128], F32R, tag="w", name="wt")
                    nc.sync.dma_start(
                        wt_first[:],
                        wq_d[dcs[0] : dcs[0] + 8, 0].rearrange("c p m -> p c m"),
                    )
                    # x tiles split into s-halves (separate tiles so the
                    # first chains unblock after half the data), alternating
                    # issue queues (SP / ACT) for parallel DMA
                    xtsA, xtsB = [], []
                    for qi, dc in enumerate(dcs):
                        xa = xt_pool.tile([128, 1024], F32R, tag="xa", name="xa")
                        eng = nc.sync if qi % 2 == 0 else nc.scalar
                        eng.dma_start(xa[:], xt_d[dc][:, 0:1024])
                        xtsA.append(xa)
                    if prio is not None:
                        prio.__exit__(None, None, None)
                    for qi, dc in enumerate(dcs):
                        xb = xt_pool.tile([128, 1024], F32R, tag="xb", name="xb")
                        eng = nc.sync if qi % 2 == 1 else nc.scalar
                        eng.dma_start(xb[:], xt_d[dc][:, 1024:2048])
                        xtsB.append(xb)

                    def xslice(i, sl_start, width):
                        # rhs slice [sl_start, sl_start+width) of logical xt[i]
                        if sl_start < 1024:
                            return xtsA[i][:, sl_start : sl_start + width]
                        return xtsB[i][:, sl_start - 1024 : sl_start - 1024 + width]
                    wvt = wv_pool.tile([128, 8, GM], F32R, tag="wv", name="wvt")
                    nc.scalar.dma_start(
                        wvt[:], wv_d[dcs[0] : dcs[0] + 8].rearrange("c p m -> p c m")
                    )
                    v_next = 0

                    def emit_v(n):
                        nonlocal v_next
                        for s in range(v_next, v_next + n):
                            ps = ps1.tile([128, GM], F32, tag="ps", name="ps")
                            for i in range(8):
                                nc.tensor.matmul(
                                    ps[:], xslice(i, s * 128, 128), wvt[:, i, :],
                                    start=(i == 0), stop=(i == 7),
                                )
                            if half == 0:
                                nc.vector.tensor_copy(out=v_t[s][:], in_=ps[:])
                            else:
                                nc.vector.tensor_add(
                                    out=v_t[s][:], in0=ps[:], in1=v_t[s][:]
                                )
                        v_next += n

                    # v-group placement: back-loaded in half 0 (wvt DMA queues
                    # behind the xt bulk); interleaved finely in half 1 so DVE
                    # rope work never outpaces PE for long
                    for h in range(G_HEADS):
                        for wi, (wd, dst) in enumerate(((wq_d, q_t[h]), (wk_d, k_t[h]))):
                            if h == 0 and wi == 0:
                                wt = wt_first
                            else:
                                wt = wqk_pool.tile([128, 8, 128], F32R, tag="w", name="wt")
                                nc.sync.dma_start(
                                    wt[:],
                                    wd[dcs[0] : dcs[0] + 8, h].rearrange("c p m -> p c m"),
                                )
                            for sc in range(SC512):
                                ps = ps1.tile([128, 512], F32, tag="ps", name="ps")
                                sl = bass.ts(sc, 512)
                                for i in range(8):
                                    nc.tensor.matmul(
                                        ps[:], wt[:, i, :], xslice(i, sc * 512, 512),
                                        start=(i == 0), stop=(i == 7),
                                    )
                                if half == 0:
                                    nc.vector.tensor_copy(out=dst[:, sl], in_=ps[:])
                                else:
                                    nc.vector.tensor_add(
                                        out=dst[:, sl], in0=ps[:], in1=dst[:, sl]
                                    )
                                    rope(dst, sl)
                        if (half == 0 and h >= 2) or half == 1:
                            emit_v({0: 8, 1: 4}[half])
                        if half == 0 and h == G_HEADS - 1:
                            # rope tables: needed from half 1 on
                            nc.scalar.dma_start(ce_t[:], ce_d)
                            nc.scalar.dma_start(s2_t[:], s2_d)

            # ---------------- phase 2: attention ----------------
            o_pool = tc.alloc_tile_pool(name="oT", bufs=1)
            o_t = [o_pool.tile([128, SEQ], F32R, tag=f"o{h}", name=f"o{h}") for h in range(G_HEADS)]
            wo_pool = tc.alloc_tile_pool(name="wo", bufs=1)
            wo_t = []
            for mc in range(G_HEADS):
                w = wo_pool.tile([128, SEQ], F32R, tag=f"wo{mc}", name=f"wo{mc}")
                nc.sync.dma_start(w[:], wo_d[mc])
                wo_t.append(w)
            with (
                tc.tile_pool(name="est", bufs=5) as est_pool,
                tc.tile_pool(name="nrm", bufs=3) as nrm_pool,
                tc.tile_pool(name="ps_st", bufs=2, space="PSUM") as ps_st,
                tc.tile_pool(name="ps_av", bufs=2, space="PSUM") as ps_av,
                tc.tile_pool(name="ps_dn", bufs=2, space="PSUM") as ps_dn,
            ):
                if with_mask:
                    mask_pool = tc.alloc_tile_pool(name="mask", bufs=2)

                for ic in range(SC512):
                    isl = bass.ts(ic, 512)
                    for hp in range(G_HEADS // 2):
                        heads = (2 * hp, 2 * hp + 1)
                        acc = {}
                        den = {}
                        e_of = {}
                        m_of = {}
                        for h in heads:
                            acc[h] = ps_av.tile([128, 512], F32, tag="acc", name="acc")
                            den[h] = ps_dn.tile([1, 512], F32, tag="den", name="den")

                        def emit_st(h, jc2):
                            ja, jb = 2 * jc2, 2 * jc2 + 1
                            st = ps_st.tile([128, 1024], F32, tag="st", name="st")
                            nc.tensor.matmul(
                                st[:, 0:512],
                                k_t[h][:, bass.ts(ja, 128)], q_t[h][:, isl],
                                start=True, stop=True,
                            )
                            nc.tensor.matmul(
                                st[:, 512:1024],
                                k_t[h][:, bass.ts(jb, 128)], q_t[h][:, isl],
                                start=True, stop=True,
                            )
                            e = est_pool.tile([128, 1024], F32R, tag="e", name="e")
                            if with_mask:
                                if jc2 not in m_of:
                                    mtl = mask_pool.tile(
                                        [128, 1024], F32, tag="m", name="mtl"
                                    )
                                    nc.sync.dma_start(mtl[:, 0:512], mt_d[ja, :, isl])
                                    nc.sync.dma_start(mtl[:, 512:1024], mt_d[jb, :, isl])
                                    m_of[jc2] = mtl
                                nc.vector.tensor_add(
                                    out=e[:], in0=st[:], in1=m_of[jc2][:]
                                )
                                nc.scalar.activation(
                                    out=e[:], in_=e[:],
                                    func=mybir.ActivationFunctionType.Exp,
                                )
                            else:
                                nc.scalar.activation(
                                    out=e[:], in_=st[:],
                                    func=mybir.ActivationFunctionType.Exp,
                                )
                            # pre-sum the two j-chunk halves on DVE so the
                            # denominator needs one PE matmul, not two
                            eh = est_pool.tile([128, 512], F32R, tag="eh", name="eh", bufs=3)
                            nc.vector.tensor_add(
                                out=eh[:], in0=e[:, 0:512], in1=e[:, 512:1024]
                            )
                            e_of[h] = (e, eh)

                        def emit_denav(h, jc2):
                            ja, jb = 2 * jc2, 2 * jc2 + 1
                            e, eh = e_of[h]
                            last = jc2 == SC128 // 2 - 1
                            nc.tensor.matmul(
                                den[h][:], ones_t[:], eh[:],
                                start=(jc2 == 0), stop=last,
                            )
                            nc.tensor.matmul(
                                acc[h][:], v_t[ja][:, bass.ts(h, 128)], e[:, 0:512],
                                start=(jc2 == 0), stop=False,
                            )
                            nc.tensor.matmul(
                                acc[h][:], v_t[jb][:, bass.ts(h, 128)], e[:, 512:1024],
                                start=False, stop=last,
                            )


                        # software pipeline: heads offset by a half step so PE
                        # always has independent matmuls while ACT runs exp
                        h0, h1 = heads
                        NJ2 = SC128 // 2
                        emit_st(h0, 0)
                        for jc2 in range(NJ2):
                            if jc2 > 0:
                                emit_st(h0, jc2)
                                emit_denav(h1, jc2 - 1)
                            emit_st(h1, jc2)
                            emit_denav(h0, jc2)
                        emit_denav(h1, NJ2 - 1)

                        for h in heads:
                            rec = nrm_pool.tile([1, 512], F32, tag="rec", name="rec")
                            nc.vector.reciprocal_approx_fast(out=rec[:], in_=den[h][:])
                            bc = nrm_pool.tile([128, 512], F32, tag="bc", name="bc")
                            nc.gpsimd.partition_broadcast(bc[:], rec[:])
                            nc.vector.tensor_mul(
                                out=o_t[h][:, isl], in0=acc[h][:], in1=bc[:]
                            )
                if with_mask:
                    mask_pool.release()

            # ---------------- phase 3: output projection ----------------
            with (
                tc.tile_pool(name="fin", bufs=10) as fin_pool,
                tc.tile_pool(name="ps3", bufs=8, space="PSUM") as ps3,
            ):
                for s in range(SC128):
                    ssl = bass.ts(s, 128)
                    for nck in range(SC512):
                        nsl = bass.ts(nck, 512)
                        ps = ps3.tile([128, 512], F32, tag="ps3", name="ps3")
                        for mc in range(G_HEADS):
                            nc.tensor.matmul(
                                ps[:], o_t[mc][:, ssl], wo_t[mc][:, nsl],
                                start=(mc == 0), stop=(mc == G_HEADS - 1),
                            )
                        f = fin_pool.tile([128, 512], F32, tag="f", name="f")
                        nc.vector.tensor_copy(out=f[:], in_=ps[:])
                        nc.sync.dma_start(out_d[s, :, nsl], f[:])
            wo_pool.release()
            o_pool.release()

    nc.compile()
    return nc


_CACHE = {}


def _get_nc(with_mask: bool):
    if with_mask not in _CACHE:
        _CACHE[with_mask] = build(with_mask)
    return _CACHE[with_mask]


def kernel(in_token, freqs_cos, freqs_sin, mask, wq, wk, wv, wo):
    return _run(in_token, freqs_cos, freqs_sin, mask, wq, wk, wv, wo)


def run_traced(in_token, freqs_cos, freqs_sin, mask, wq, wk, wv, wo):
    """Test-only: run with NTFF tracing, return (output, BassKernelResults)."""
    return _run(in_token, freqs_cos, freqs_sin, mask, wq, wk, wv, wo, trace=True)


def _run(in_token, freqs_cos, freqs_sin, mask, wq, wk, wv, wo, trace=False):
    in_token = np.ascontiguousarray(np.asarray(in_token, dtype=np.float32))
    freqs_cos = np.asarray(freqs_cos, dtype=np.float32)
    freqs_sin = np.asarray(freqs_sin, dtype=np.float32)
    mask = np.asarray(mask, dtype=np.float32)
    wq = np.asarray(wq, dtype=np.float32)
    wk = np.asarray(wk, dtype=np.float32)
    wv = np.asarray(wv, dtype=np.float32)
    wo = np.asarray(wo, dtype=np.float32)

    with_mask = bool(np.any(mask))
    nc = _get_nc(with_mask)

    # rope tables in (head_dim, seq) pair-expanded layout, signs/swap baked in
    ce = np.repeat(freqs_cos.T, 2, axis=0).astype(np.float32)  # (128, S)
    s2 = np.empty((HEAD_DIM, SEQ), np.float32)
    s2[0::2] = freqs_sin.T   # even rows: +sin (lands on odd out after swap)
    s2[1::2] = -freqs_sin.T  # odd rows: -sin (lands on even out after swap)
    ones = np.ones((128, 1), np.float32)
    if with_mask:
        mt = np.ascontiguousarray(mask.T).reshape(SC128, 128, SEQ)

    in_maps = []
    xts = [
        np.ascontiguousarray(in_token[b].T).reshape(DC, 128, SEQ)
        for b in range(BATCH)
    ]
    for b in range(BATCH):
        for g in range(G_HEADS):
            rows = slice(g * GM, (g + 1) * GM)
            wqt = np.ascontiguousarray(
                (wq[rows] * INV_SQRT_HD).T.reshape(
                    DC, 128, G_HEADS, 128
                ).transpose(0, 2, 1, 3)
            )
            wkt = np.ascontiguousarray(
                wk[rows].T.reshape(DC, 128, G_HEADS, 128).transpose(0, 2, 1, 3)
            )
            wvt = np.ascontiguousarray(wv[rows].T).reshape(DC, 128, GM)
            wot = np.ascontiguousarray(wo[:, rows].T).reshape(G_HEADS, 128, SEQ)
            m = {
                "xt": xts[b], "wq": wqt, "wk": wkt, "wv": wvt, "wo": wot,
                "ce": ce, "s2": s2, "ones": ones,
            }
            if with_mask:
                m["mt"] = mt
            in_maps.append(m)

    res = run_bass_kernel_spmd(nc, in_maps, core_ids=list(range(8)), trace=trace)

    out = np.zeros((BATCH, SEQ, DIM), np.float32)
    for b in range(BATCH):
        acc = None
        for g in range(G_HEADS):
            p = res.results[b * G_HEADS + g]["out"].reshape(SEQ, DIM)
            acc = p if acc is None else acc + p
        out[b] = acc
    if trace:
        return out, res
    return out



# revision 3
# speedup vs baseline: 1.0012x; 1.0012x over previous
"""Multi-head attention (B=2, S=2048, D=2048, H=16) on 8 Trainium2 cores.

Sharding: core = batch (2) x head-group (4 heads each). Tensor-parallel on
wq/wk/wv rows + wo columns; per-core partial outputs summed on host.

Precision plan (rel_l2 budget 2e-2, measured ~4e-3):
  - q/k/v projections + wo: fp8e4 DoubleRow matmuls with 3-term hi/lo
    residual splits (x ~ xh+xl, w ~ wh+wl; compute xh*wh + xh*wl + xl*wh).
    4x fewer PE cycles per term than fp32r => 0.75x total.
  - scores / attn@V: bf16 (same PE cost as fp32r, half the SBUF/DVE cost).
  - softmax denominator: DVE bf16 pairwise tree + gpsimd partition
    all-reduce (no PE cycles, no PSUM banks).
  - wo matmuls for seq-chunk ic interleaved into attention of ic+1.
"""

import sys

for _p in ("/opt/trn_rl_repo",):
    if _p not in sys.path:
        sys.path.insert(0, _p)

import numpy as np
import ml_dtypes

import concourse.bass as bass
import concourse.bass_isa as bass_isa
import concourse.tile as tile
from concourse import bacc, mybir
from concourse.bass_utils import run_bass_kernel_spmd

F32 = mybir.dt.float32
BF16 = mybir.dt.bfloat16
FP8 = mybir.dt.float8e4
DR = mybir.MatmulPerfMode.DoubleRow
NP_FP8 = ml_dtypes.float8_e4m3
NP_BF16 = ml_dtypes.bfloat16

DIM = 2048
N_HEADS = 16
HEAD_DIM = 128
BATCH = 2
SEQ = 2048
G_HEADS = 4              # heads per core
GM = G_HEADS * HEAD_DIM  # 512 output cols per core
DC = DIM // 128          # 16 contraction chunks
SC512 = SEQ // 512       # 4
SC128 = SEQ // 128       # 16
SQD = float(np.sqrt(DIM))
# exp scale: st = (sqrt(D) q) . (sqrt(D) k) = D * (q.k); true scores = q.k/sqrt(hd)
ALPHA = float(1.0 / (DIM * np.sqrt(HEAD_DIM)))
# final out scale: ps3 = (sqrt(D) o) . (sqrt(D) wo) = D * out
OUT_SCALE = float(1.0 / DIM)

# even<->odd partition swap within each 32-partition group
_SWAP_MASK = [i ^ 1 for i in range(32)]


def build(with_mask: bool):
    nc = bacc.Bacc("TRN2", target_bir_lowering=False, debug=False)

    xh_d = nc.dram_tensor("xh", [128, DC, SEQ], FP8, kind="ExternalInput").ap()
    xl_d = nc.dram_tensor("xl", [128, DC, SEQ], FP8, kind="ExternalInput").ap()
    wq_d = {}
    for nm in ("wqh", "wql", "wkh", "wkl"):
        wq_d[nm] = nc.dram_tensor(nm, [128, DC, GM], FP8, kind="ExternalInput").ap()
    wvh_d = nc.dram_tensor("wvh", [128, DC, GM], FP8, kind="ExternalInput").ap()
    wvl_d = nc.dram_tensor("wvl", [128, DC, GM], FP8, kind="ExternalInput").ap()
    woh_d = nc.dram_tensor("woh", [128, G_HEADS, SEQ], FP8, kind="ExternalInput").ap()
    wol_d = nc.dram_tensor("wol", [128, G_HEADS, SEQ], FP8, kind="ExternalInput").ap()
    ce_d = nc.dram_tensor("ce", [128, SEQ], BF16, kind="ExternalInput").ap()
    s2_d = nc.dram_tensor("s2", [128, SEQ], BF16, kind="ExternalInput").ap()
    mt_d = None
    if with_mask:
        mt_d = nc.dram_tensor("mt", [SC128, 128, SEQ], F32, kind="ExternalInput").ap()
    out_d = nc.dram_tensor("out", [SC128, 128, SEQ], BF16, kind="ExternalOutput").ap()

    with tile.TileContext(nc) as tc:
        with (
            tc.tile_pool(name="persist", bufs=1) as persist,
            tc.tile_pool(name="consts", bufs=1) as consts,
        ):
            # warm the ACT exp LUT early so phase 2 doesn't stall on it
            warm_t = consts.tile([128, 1], F32, tag="warm")
            nc.vector.memset(warm_t[:], 1.0)
            warm2_t = consts.tile([128, 1], F32, tag="warm2")
            nc.scalar.activation(
                out=warm2_t[:], in_=warm_t[:],
                func=mybir.ActivationFunctionType.Exp,
            )

            q_t = [persist.tile([128, SEQ], BF16, tag=f"q{h}", name=f"q{h}") for h in range(G_HEADS)]
            k_t = [persist.tile([128, SEQ], BF16, tag=f"k{h}", name=f"k{h}") for h in range(G_HEADS)]
            v_t = persist.tile([128, DC, GM], BF16, tag="v", name="v_t")
            # o tiles: pair layout for DoubleRow wo (heads (0,1) and (2,3))
            o_hi = [persist.tile([128, 2, SEQ], FP8, tag=f"oh{p}", name=f"oh{p}") for p in range(2)]
            o_lo = [persist.tile([128, 2, SEQ], FP8, tag=f"ol{p}", name=f"ol{p}") for p in range(2)]
            ce_t = persist.tile([128, SEQ], BF16, tag="ce", name="ce_t")
            s2_t = persist.tile([128, SEQ], BF16, tag="s2", name="s2_t")
            wo_t = {
                "h": persist.tile([128, G_HEADS, SEQ], FP8, tag="woh", name="wo_h"),
                "l": persist.tile([128, G_HEADS, SEQ], FP8, tag="wol", name="wo_l"),
            }

            # ---------------- phase 1: projections + fused rope ----------------
            with (
                tc.tile_pool(name="xp", bufs=1) as xp_pool,
                tc.tile_pool(name="wv", bufs=1) as wv_pool,
                tc.tile_pool(name="wqk", bufs=4) as wqk_pool,
                tc.tile_pool(name="ps1", bufs=8, space="PSUM") as ps1,
                tc.tile_pool(name="rope_t", bufs=2) as rope_t,
            ):
                # x hi/lo resident for the whole phase (4 MB fp8 each)
                xh_t = xp_pool.tile([128, DC, SEQ], FP8, tag="xh", name="xh_t")
                xl_t = xp_pool.tile([128, DC, SEQ], FP8, tag="xl", name="xl_t")
                # The cost model serializes all DMA transfers through one
                # shared DMA_ENGINES device (~360 GB/s), so what matters is
                # the GLOBAL transfer order, not queue parallelism. Stream
                # inputs chunk-pair-major on SP, matching the K-major order
                # the chains consume them in, so every open psum group makes
                # progress as each pair lands. Weight pair-slices keep a 512B
                # inner run (no <512B DMA penalty). Stragglers (rope tables,
                # wo) go on gpsimd with delayed priority.
                w_tiles = {}
                for nm in ("wqh", "wql", "wkh", "wkl"):
                    w_tiles[nm] = wqk_pool.tile(
                        [128, DC, GM], FP8, tag="w", name="wt"
                    )
                wv_h = wv_pool.tile([128, DC, GM], FP8, tag="wvh", name="wv_h")
                wv_l = wv_pool.tile([128, DC, GM], FP8, tag="wvl", name="wv_l")
                with tc.high_priority():
                    for c in range(2):
                        nc.sync.dma_start(xh_t[:, c, :], xh_d[:, c, :])
                    for c2 in range(4):
                        psl = slice(2 * c2, 2 * c2 + 2)
                        nc.sync.dma_start(
                            w_tiles["wqh"][:, psl, :], wq_d["wqh"][:, psl, :]
                        )
                        nc.sync.dma_start(
                            w_tiles["wql"][:, psl, :], wq_d["wql"][:, psl, :]
                        )
                for c in range(2, 8):
                    nc.sync.dma_start(xh_t[:, c, :], xh_d[:, c, :])
                for c2 in range(4, DC // 2):
                    psl = slice(2 * c2, 2 * c2 + 2)
                    nc.sync.dma_start(
                        w_tiles["wqh"][:, psl, :], wq_d["wqh"][:, psl, :]
                    )
                    nc.sync.dma_start(
                        w_tiles["wql"][:, psl, :], wq_d["wql"][:, psl, :]
                    )
                for c in range(8, DC):
                    nc.sync.dma_start(xh_t[:, c, :], xh_d[:, c, :])
                nc.sync.dma_start(w_tiles["wkh"][:], wq_d["wkh"])
                nc.sync.dma_start(w_tiles["wkl"][:], wq_d["wkl"])
                for c in range(DC):
                    nc.sync.dma_start(xl_t[:, c, :], xl_d[:, c, :])
                nc.sync.dma_start(wv_h[:], wvh_d)
                nc.sync.dma_start(wv_l[:], wvl_d)

                def rope(t, sl):
                    # q' = t*ce + swap(t*s2); sin-product on gpsimd
                    t1 = rope_t.tile([128, 512], BF16, tag="t1", name="t1")
                    nc.gpsimd.tensor_mul(out=t1[:], in0=t[:, sl], in1=s2_t[:, sl])
                    t3 = rope_t.tile([128, 512], BF16, tag="t3", name="t3")
                    nc.vector.tensor_mul(out=t3[:], in0=t[:, sl], in1=ce_t[:, sl])
                    t2 = rope_t.tile([128, 512], BF16, tag="t2", name="t2")
                    nc.vector.stream_shuffle(t2[:], t1[:], _SWAP_MASK)
                    nc.vector.tensor_add(out=t[:, sl], in0=t3[:], in1=t2[:])

                def proj_chain(ps, wh, wl, col_sl, x_sl):
                    """24 chained DoubleRow MMs: xh*wh + xh*wl + xl*wh;
                    xl term last so hi-terms run while xl streams in."""
                    n_mm = 3 * (DC // 2)
                    i = 0
                    for xt, wt in ((xh_t, wh), (xh_t, wl), (xl_t, wh)):
                        for c2 in range(DC // 2):
                            nc.tensor.matmul(
                                ps[:],
                                wt[:, 2 * c2 : 2 * c2 + 2, col_sl],
                                xt[:, 2 * c2 : 2 * c2 + 2, x_sl],
                                start=(i == 0),
                                stop=(i == n_mm - 1),
                                perf_mode=DR,
                            )
                            i += 1

                def v_chain(ps, col_sl, x_sl):
                    """v: x is stationary, wv moving."""
                    n_mm = 3 * (DC // 2)
                    i = 0
                    for xt, wt in ((xh_t, wv_h), (xh_t, wv_l), (xl_t, wv_h)):
                        for c2 in range(DC // 2):
                            nc.tensor.matmul(
                                ps[:],
                                xt[:, 2 * c2 : 2 * c2 + 2, x_sl],
                                wt[:, 2 * c2 : 2 * c2 + 2, col_sl],
                                start=(i == 0),
                                stop=(i == n_mm - 1),
                                perf_mode=DR,
                            )
                            i += 1

                v_next = 0

                def emit_v(n):
                    nonlocal v_next
                    for s in range(v_next, v_next + n):
                        ps = ps1.tile([128, GM], F32, tag="ps", name="ps")
                        v_chain(ps, slice(0, GM), bass.ts(s, 128))
                        if s >= 10:
                            # late copies on DVE: keeps ACT's queue clear so
                            # the first attention exps start without waiting
                            # behind phase-1 copy stragglers
                            nc.vector.tensor_copy(out=v_t[:, s, :], in_=ps[:])
                        else:
                            nc.scalar.copy(out=v_t[:, s, :], in_=ps[:])
                    v_next += n

                # v weights right behind xl on the SP stream
                nc.sync.dma_start(wv_h[:], wvh_d)
                nc.sync.dma_start(wv_l[:], wvl_d)

                def qk_chain(h, wi, sc):
                    hi_nm, lo_nm, dst = (
                        ("wqh", "wql", q_t[h]) if wi == 0 else ("wkh", "wkl", k_t[h])
                    )
                    ps = ps1.tile([128, 512], F32, tag="ps", name="ps")
                    sl = bass.ts(sc, 512)
                    proj_chain(ps, w_tiles[hi_nm], w_tiles[lo_nm], bass.ts(h, 128), sl)
                    nc.scalar.copy(out=dst[:, sl], in_=ps[:])
                    rope(dst, sl)

                for h in range(G_HEADS):
                    for wi in range(2):
                        for sc in range(SC512):
                            qk_chain(h, wi, sc)
                    if h >= 1:
                        emit_v(5 if h < G_HEADS - 1 else 6)
                # stragglers at the tail of the SP stream: one queue keeps
                # strict order, so these can't jump ahead of the critical path
                nc.sync.dma_start(ce_t[:], ce_d)
                nc.sync.dma_start(s2_t[:], s2_d)
                nc.sync.dma_start(wo_t["h"][:], woh_d)
                nc.sync.dma_start(wo_t["l"][:], wol_d)

            # ---------------- phase 2: attention + fused wo ----------------
            with (
                tc.tile_pool(name="est", bufs=6) as est_pool,
                tc.tile_pool(name="ehp", bufs=10) as eh_pool,
                tc.tile_pool(name="nrm", bufs=4) as nrm_pool,
                tc.tile_pool(name="fin", bufs=6) as fin_pool,
                tc.tile_pool(name="ps_st", bufs=2, space="PSUM") as ps_st,
                tc.tile_pool(name="ps_av", bufs=3, space="PSUM") as ps_av,
                tc.tile_pool(name="ps_wo", bufs=1, space="PSUM") as ps_wo,
            ):
                if with_mask:
                    mask_pool = tc.alloc_tile_pool(name="mask", bufs=2)

                # wo tile-chain generator for seq-chunk ic; emitted lazily,
                # interleaved with the next ic's attention matmuls
                def wo_tiles(ic):
                    # the last ic's drain runs after attention ends, so it
                    # borrows the (then-idle) ps_av banks for a deeper buffer
                    pool = ps_av if ic == SC512 - 1 else ps_wo
                    tag = "acc" if ic == SC512 - 1 else "ps3"
                    for s in range(4 * ic, 4 * ic + 4):
                        ssl = bass.ts(s, 128)
                        for nck in range(SC512):
                            nsl = bass.ts(nck, 512)
                            ps = pool.tile([128, 512], F32, tag=tag, name="ps3")
                            i = 0
                            terms = (
                                (o_hi, wo_t["h"]),
                                (o_hi, wo_t["l"]),
                                (o_lo, wo_t["h"]),
                            )
                            n_mm = len(terms) * 2
                            for ot, wt in terms:
                                for p in range(2):
                                    nc.tensor.matmul(
                                        ps[:],
                                        ot[p][:, :, ssl],
                                        wt[:, 2 * p : 2 * p + 2, nsl],
                                        start=(i == 0),
                                        stop=(i == n_mm - 1),
                                        perf_mode=DR,
                                    )
                                    i += 1
                            f = fin_pool.tile([128, 512], BF16, tag="f", name="f")
                            # gpsimd cannot touch PSUM on HW. DVE-only while
                            # attention runs; ACT joins for the final drain
                            # (its exp stream is done by then)
                            if ic == SC512 - 1 and (s + nck) % 2 == 1:
                                nc.scalar.mul(out=f[:], in_=ps[:], mul=OUT_SCALE)
                            else:
                                nc.vector.tensor_scalar_mul(
                                    out=f[:], in0=ps[:], scalar1=OUT_SCALE
                                )
                            nc.sync.dma_start(out_d[s, :, nsl], f[:])
                            yield

                # Rolling software pipeline over all 16 (ic, head) blocks.
                # av lags st by 2 slots, giving each exp ~2.3us of slack; no
                # per-pair drain/refill boundaries.
                wo_gen = None
                wo_phase = 0
                NJ2 = SC128 // 2
                blocks = [(ic, h) for ic in range(SC512) for h in range(G_HEADS)]
                acc = {}
                e_of = {}
                eh_of = {}

                def emit_st(bi, jc2):
                    ic, h = blocks[bi]
                    isl = bass.ts(ic, 512)
                    ja, jb = 2 * jc2, 2 * jc2 + 1
                    st = ps_st.tile([128, 1024], F32, tag="st", name="st")
                    nc.tensor.matmul(
                        st[:, 0:512],
                        k_t[h][:, bass.ts(ja, 128)], q_t[h][:, isl],
                        start=True, stop=True,
                    )
                    nc.tensor.matmul(
                        st[:, 512:1024],
                        k_t[h][:, bass.ts(jb, 128)], q_t[h][:, isl],
                        start=True, stop=True,
                    )
                    e = est_pool.tile([128, 1024], BF16, tag="e", name="e")
                    if with_mask:
                        mtl = mask_pool.tile([128, 1024], F32, tag="m", name="mtl")
                        nc.sync.dma_start(mtl[:, 0:512], mt_d[ja, :, isl])
                        nc.sync.dma_start(mtl[:, 512:1024], mt_d[jb, :, isl])
                        nc.vector.tensor_scalar(
                            out=st[:], in0=st[:], scalar1=ALPHA,
                            op0=mybir.AluOpType.mult,
                        )
                        nc.vector.tensor_add(out=st[:], in0=st[:], in1=mtl[:])
                        nc.scalar.activation(
                            out=e[:], in_=st[:],
                            func=mybir.ActivationFunctionType.Exp,
                        )
                    else:
                        nc.scalar.activation(
                            out=e[:], in_=st[:],
                            func=mybir.ActivationFunctionType.Exp,
                            scale=ALPHA,
                        )
                    e_of[(bi, jc2)] = e

                def emit_av(bi, jc2):
                    ic, h = blocks[bi]
                    if jc2 == 0:
                        acc[bi] = ps_av.tile([128, 512], F32, tag="acc", name="acc")
                        eh_of[bi] = []
                    ja, jb = 2 * jc2, 2 * jc2 + 1
                    e = e_of.pop((bi, jc2))
                    last = jc2 == NJ2 - 1
                    nc.tensor.matmul(
                        acc[bi][:], v_t[:, ja, bass.ts(h, 128)], e[:, 0:512],
                        start=(jc2 == 0), stop=False,
                    )
                    nc.tensor.matmul(
                        acc[bi][:], v_t[:, jb, bass.ts(h, 128)], e[:, 512:1024],
                        start=False, stop=last,
                    )
                    # denominator partial: bf16 pairwise tree level 0,
                    # streamed (combine equal-level partials eagerly)
                    eh = eh_pool.tile([128, 512], BF16, tag="eh", name="eh")
                    nc.vector.tensor_add(
                        out=eh[:], in0=e[:, 0:512], in1=e[:, 512:1024]
                    )
                    stack = eh_of[bi]
                    lvl = 0
                    while stack and stack[-1][0] == lvl:
                        _, prev = stack.pop()
                        nxt = eh_pool.tile([128, 512], BF16, tag="eh", name="ehs")
                        nc.vector.tensor_add(out=nxt[:], in0=prev[:], in1=eh[:])
                        eh = nxt
                        lvl += 1
                    stack.append((lvl, eh))
                    if last:
                        finish(bi)

                def finish(bi):
                    ic, h = blocks[bi]
                    isl = bass.ts(ic, 512)
                    stack = eh_of.pop(bi)
                    esum = stack.pop()[1]
                    while stack:
                        _, prev = stack.pop()
                        s = eh_pool.tile([128, 512], BF16, tag="eh", name="ehd")
                        nc.vector.tensor_add(out=s[:], in0=prev[:], in1=esum[:])
                        esum = s
                    den = nrm_pool.tile([128, 512], F32, tag="den", name="den")
                    nc.gpsimd.partition_all_reduce(
                        den[:], esum[:], 128, bass_isa.ReduceOp.add
                    )
                    rec = nrm_pool.tile([128, 512], F32, tag="rec", name="rec")
                    nc.vector.reciprocal_approx_fast(out=rec[:], in_=den[:])
                    t = nrm_pool.tile([128, 512], F32, tag="t", name="t")
                    nc.vector.tensor_mul(out=t[:], in0=acc.pop(bi)[:], in1=rec[:])
                    # o hi/lo quantize, alternating DVE/gpsimd; the final
                    # block stays on DVE (faster) since the wo(3) drain waits
                    # directly on it
                    pi, off = divmod(h, 2)
                    use_dve = h % 2 == 0 or bi == len(blocks) - 1
                    eng = nc.vector if use_dve else nc.gpsimd
                    eng.tensor_copy(out=o_hi[pi][:, off, isl], in_=t[:])
                    eng.tensor_sub(
                        out=o_lo[pi][:, off, isl], in0=t[:],
                        in1=o_hi[pi][:, off, isl],
                    )

                def step_wo():
                    # one wo tile every other slot: 16 tiles spread evenly
                    # over each ic's 32 slots
                    nonlocal wo_gen, wo_phase
                    wo_phase += 1
                    if wo_gen is not None and wo_phase % 2 == 0:
                        next(wo_gen, None)

                LAG = 2
                slot = 0
                for bi in range(len(blocks)):
                    ic, h = blocks[bi]
                    for jc2 in range(NJ2):
                        emit_st(bi, jc2)
                        k = slot - LAG
                        if k >= 0:
                            emit_av(k // NJ2, k % NJ2)
                        step_wo()
                        slot += 1
                    if h == G_HEADS - 1:
                        # all of this ic's avs may still be in flight, but
                        # wo pieces wait on o via semaphores anyway
                        wo_gen = wo_tiles(ic)
                for k in range(slot - LAG, slot):
                    emit_av(k // NJ2, k % NJ2)
                    step_wo()
                # drain the last ic's wo chains
                for _ in wo_gen:
                    pass
                if with_mask:
                    mask_pool.release()

    nc.compile()
    return nc


_CACHE = {}


def _get_nc(with_mask: bool):
    if with_mask not in _CACHE:
        _CACHE[with_mask] = build(with_mask)
    return _CACHE[with_mask]


def _split8(a):
    hi = np.asarray(a, NP_FP8)
    lo = np.asarray(a - hi.astype(np.float32), NP_FP8)
    return hi, lo


def kernel(in_token, freqs_cos, freqs_sin, mask, wq, wk, wv, wo):
    return _run(in_token, freqs_cos, freqs_sin, mask, wq, wk, wv, wo)


def run_traced(in_token, freqs_cos, freqs_sin, mask, wq, wk, wv, wo):
    """Test-only: run with NTFF tracing, return (output, BassKernelResults)."""
    return _run(in_token, freqs_cos, freqs_sin, mask, wq, wk, wv, wo, trace=True)


def _run(in_token, freqs_cos, freqs_sin, mask, wq, wk, wv, wo, trace=False):
    in_token = np.asarray(in_token, dtype=np.float32)
    freqs_cos = np.asarray(freqs_cos, dtype=np.float32)
    freqs_sin = np.asarray(freqs_sin, dtype=np.float32)
    mask = np.asarray(mask, dtype=np.float32)
    wq = np.asarray(wq, dtype=np.float32)
    wk = np.asarray(wk, dtype=np.float32)
    wv = np.asarray(wv, dtype=np.float32)
    wo = np.asarray(wo, dtype=np.float32)

    with_mask = bool(np.any(mask))
    nc = _get_nc(with_mask)

    # rope tables in (head_dim, seq) pair-expanded layout, signs/swap baked in
    ce = np.repeat(freqs_cos.T, 2, axis=0)
    s2 = np.empty((HEAD_DIM, SEQ), np.float32)
    s2[0::2] = freqs_sin.T   # even rows: +sin (lands on odd out after swap)
    s2[1::2] = -freqs_sin.T  # odd rows: -sin (lands on even out after swap)
    ce = np.asarray(ce, NP_BF16)
    s2 = np.asarray(s2, NP_BF16)
    if with_mask:
        mt = np.ascontiguousarray(mask.T).reshape(SC128, 128, SEQ)

    in_maps = []
    # x transposed to [xd, seq], chunked: [128, DC, SEQ]
    xs = []
    for b in range(BATCH):
        xt = np.ascontiguousarray(
            in_token[b].T.reshape(DC, 128, SEQ).transpose(1, 0, 2)
        )
        xs.append(_split8(xt))
    for b in range(BATCH):
        xh, xl = xs[b]
        for g in range(G_HEADS):
            rows = slice(g * GM, (g + 1) * GM)

            def wlay(w):
                # [out 512, in 2048] -> [128, DC, 512]
                return np.ascontiguousarray(
                    (w * SQD).T.reshape(DC, 128, GM).transpose(1, 0, 2)
                )

            wqh, wql = _split8(wlay(wq[rows]))
            wkh, wkl = _split8(wlay(wk[rows]))
            wvh, wvl = _split8(wlay(wv[rows]))
            # wo rows for this group: [2048 out, 512 o] -> [128, 4, 2048]
            wog = np.ascontiguousarray(
                (wo[:, rows] * SQD).T.reshape(G_HEADS, 128, SEQ).transpose(1, 0, 2)
            )
            woh, wol = _split8(wog)
            m = {
                "xh": xh, "xl": xl,
                "wqh": wqh, "wql": wql, "wkh": wkh, "wkl": wkl,
                "wvh": wvh, "wvl": wvl, "woh": woh, "wol": wol,
                "ce": ce, "s2": s2,
            }
            if with_mask:
                m["mt"] = mt
            in_maps.append(m)

    res = run_bass_kernel_spmd(nc, in_maps, core_ids=list(range(8)), trace=trace)

    out = np.zeros((BATCH, SEQ, DIM), np.float32)
    for b in range(BATCH):
        acc = None
        for g in range(G_HEADS):
            p = res.results[b * G_HEADS + g]["out"].astype(np.float32).reshape(SEQ, DIM)
            acc = p if acc is None else acc + p
        out[b] = acc
    if trace:
        return out, res
    return out
